# revision 1
# baseline (speedup 1.0000x reference)
"""Deformable-DETR transformer encoder layer on 8 Trainium2 NeuronCores.

Sharding: data-parallel over batch (B=2 -> 4 cores per batch element),
sequence-parallel over queries within the batch group. Each core computes
the full value memory for its batch element, stores it to DRAM in a
4-corner "pair" layout (bf16, 128 elems = 256B per chunk: both x corners
of both y rows of a bilinear sample), then gathers one chunk per
(head, level, point) via SWDGE dma_gather preps spread over 4 queues.

Self-contained: hardcodes all shapes/constants from the problem spec.
"""

import numpy as np
import ml_dtypes

import concourse.bass as bass
import concourse.mybir as mybir
import concourse.tile as tile
from concourse import bacc
from concourse.bass_utils import run_bass_kernel_spmd

F32 = mybir.dt.float32
I32 = mybir.dt.int32
I16 = mybir.dt.int16
BF16 = mybir.dt.bfloat16

# ---- problem constants -------------------------------------------------
SPATIAL = [(100, 100), (50, 50), (25, 25), (13, 13)]
LEVEL_START = [0, 10000, 12500, 13125]
LEN = 13294
D = 256
NH = 8
NL = 4
NP = 4
DH = 32
DFF = 1024
EPS = 1e-5

PAD_LEN = 13312           # 104 * 128, full-sequence padded length
N_FULL_TILES = PAD_LEN // 128
Q_SH = 3328               # 26 * 128, per-core query shard (padded)
N_Q_TILES = Q_SH // 128

FP = 104                  # front pad rows per head stripe (>= max W)
STRIPE = FP + PAD_LEN + 1  # 13417 rows per head stripe
CHUNK = 4 * DH            # 128 elems per chunk: (y0,y1) x (x0,x1) x DH
NT = NH * NL * NP         # 128 (h,l,p) triples = chunks per query
NIDX = 32 * 128           # idxs per gather (2 heads x 16 lp x 128 queries)

TWO23 = float(3 << 22)  # 1.5*2^23 magic round constant

# level-of-row segments for a 128-row tile: (start, n, W)
_LEVEL_BOUNDS = [(0, 100), (10000, 50), (12500, 25), (13125, 13)]


def _tile_segs(i):
    g0, g1 = i * 128, (i + 1) * 128
    segs = []
    for k, (ls, w) in enumerate(_LEVEL_BOUNDS):
        le = _LEVEL_BOUNDS[k + 1][0] if k + 1 < len(_LEVEL_BOUNDS) else PAD_LEN
        s, e = max(g0, ls), min(g1, le)
        if s < e:
            segs.append((s - g0, e - s, w))
    return segs


def _ap(t, offset_elems, dims):
    """Custom free-dim AP view of an SBUF tile (keeps full partition dim)."""
    base = t[:]
    return bass.AP(base.tensor, base.offset + offset_elems, [list(base.ap[0])] + [list(d) for d in dims])


def _apb(base, offset_elems, dims):
    """Custom free-dim AP from a pre-sliced base AP (partition sub-range)."""
    return bass.AP(base.tensor, base.offset + offset_elems, [list(base.ap[0])] + [list(d) for d in dims])


def build(dbg=False, ablate=()):
    nc = bacc.Bacc("TRN2", target_bir_lowering=False, debug=False, num_devices=8,
                   num_swdge_queues=4, dynamic_dma_scratch_size=40960)
    A = mybir.AluOpType
    ACTF = mybir.ActivationFunctionType

    def param(name, shape, dtype=F32, out=False):
        return nc.declare_dram_parameter(name, list(shape), dtype, isOutput=out)

    src_full = param("src_full", [PAD_LEN, D])
    srcq = param("srcq", [Q_SH, D])
    posq = param("posq", [Q_SH, D])
    refq = param("refq", [Q_SH, NL * 2])
    value_t = param("value_t", [NH * STRIPE, 2 * DH], BF16)
    value2 = param("value2", [NH * STRIPE, CHUNK], BF16)
    Wv = param("Wv", [D, D])
    Woa = param("Woa", [D, D + NT])
    Wout = param("Wout", [D, D])
    W1 = param("W1", [D, DFF])
    W2 = param("W2", [DFF, D])
    bv = param("bv", [1, D])
    boa = param("boa", [1, D + NT])
    bout = param("bout", [1, D])
    b1T = param("b1T", [128, DFF // 128])
    b2 = param("b2", [1, D])
    g1r = param("g1r", [128, D])
    be1r = param("be1r", [128, D])
    g2r = param("g2r", [128, D])
    be2r = param("be2r", [128, D])
    ident = param("ident", [128, 128])
    ones_row = param("ones_row", [1, 128])
    cW = param("cW", [128, NT])
    cH = param("cH", [128, NT])
    cWm1 = param("cWm1", [128, NT])
    cHm1 = param("cHm1", [128, NT])
    cWm2 = param("cWm2", [128, NT])
    cHm2 = param("cHm2", [128, NT])
    cBASE = param("cBASE", [128, NT])
    dims8 = param("dims8", [128, NL * 2])
    outq = param("outq", [Q_SH, D], out=True)
    if dbg:
        d_px = param("d_px", [Q_SH, D], out=True)
        d_aw = param("d_aw", [Q_SH, NT], out=True)
        d_w4 = param("d_w4", [Q_SH, 4 * NT], out=True)
        d_ofs = param("d_ofs", [Q_SH, NT], out=True)
        d_samp = param("d_samp", [Q_SH, D], out=True)

    with tile.TileContext(nc) as tc:
        gsems = [nc.alloc_semaphore(f"gsem{t}") for t in range(4)]
        with (
            tc.tile_pool(name="const", bufs=1) as cp,
        ):
            v2b = value2[:]

            def cload(src_ap, p, n, tag):
                t = cp.tile([p, n], F32, tag=tag)
                nc.sync.dma_start(t[:], src_ap[:])
                return t

            tWv = cp.tile([128, 2 * D], F32, tag="tWv")
            nc.sync.dma_start(tWv[:, 0:D], Wv[0:128, :])
            nc.sync.dma_start(tWv[:, D:2 * D], Wv[128:256, :])
            DOA = D + NT
            tWoa = cp.tile([128, 2 * DOA], F32, tag="tWoa")
            nc.sync.dma_start(tWoa[:, 0:DOA], Woa[0:128, :])
            nc.sync.dma_start(tWoa[:, DOA:2 * DOA], Woa[128:256, :])
            tWout = cp.tile([128, 2 * D], F32, tag="tWout")
            nc.sync.dma_start(tWout[:, 0:D], Wout[0:128, :])
            nc.sync.dma_start(tWout[:, D:2 * D], Wout[128:256, :])
            tW1 = cp.tile([128, 2 * DFF], F32, tag="tW1")
            nc.sync.dma_start(tW1[:, 0:DFF], W1[0:128, :])
            nc.sync.dma_start(tW1[:, DFF:2 * DFF], W1[128:256, :])
            tW2 = cp.tile([128, 8 * D], F32, tag="tW2")
            for j in range(8):
                nc.sync.dma_start(tW2[:, j * D:(j + 1) * D], W2[j * 128:(j + 1) * 128, :])

            tbv = cload(bv, 1, D, "tbv")
            tboa = cload(boa, 1, DOA, "tboa")
            tbout = cload(bout, 1, D, "tbout")
            tb1T = cload(b1T, 128, DFF // 128, "tb1T")
            tb2 = cload(b2, 1, D, "tb2")
            tg1 = cload(g1r, 128, D, "tg1")
            tbe1 = cload(be1r, 128, D, "tbe1")
            tg2 = cload(g2r, 128, D, "tg2")
            tbe2 = cload(be2r, 128, D, "tbe2")
            tid = cload(ident, 128, 128, "tid")
            tones = cload(ones_row, 1, 128, "tones")
            tcW = cload(cW, 128, NT, "tcW")
            tcH = cload(cH, 128, NT, "tcH")
            tcWm1 = cload(cWm1, 128, NT, "tcWm1")
            tcHm1 = cload(cHm1, 128, NT, "tcHm1")
            tcWm2 = cload(cWm2, 128, NT, "tcWm2")
            tcHm2 = cload(cHm2, 128, NT, "tcHm2")
            tcBASE = cload(cBASE, 128, NT, "tcBASE")
            tdims8 = cload(dims8, 128, NL * 2, "tdims8")

            # small scalar constants for ACT bias operands
            def cconst(val, tag):
                t = cp.tile([128, 1], F32, tag=tag)
                nc.vector.memset(t[:], val)
                return t

            t23 = cconst(TWO23, "t23")
            tm23 = cconst(-TWO23, "tm23")
            tone1 = cconst(1.0, "tone1")
            teps = cconst(EPS, "teps")

            # ---------------- Phase A: value projection ----------------
            # Pass 1: x-pair table value_t[r, 0:32] = v[r-FP-1],
            # [32:64] = v[r-FP] via the 64-el row-spill run (row width 64).
            # Pass 2 (DRAM->DRAM): value2[r, 0:64] = value_t[r, :];
            # value2[r, 64:128] = value_t[r + W_level(r), :].
            vtb = value_t[:]
            with (
                tc.tile_pool(name="pA", bufs=4) as pA,
                tc.tile_pool(name="psA", bufs=4, space="PSUM") as psA,
            ):
                for i in range(0 if "noa" in ablate else N_FULL_TILES):
                    rs = slice(i * 128, (i + 1) * 128)
                    s = pA.tile([128, D], F32, tag="As")
                    nc.sync.dma_start(s[:], src_full[rs, :])
                    sT = pA.tile([128, 2, 128], F32, tag="AsT")
                    for k in range(2):
                        tp = psA.tile([128, 128], F32, tag="Atp")
                        nc.tensor.transpose(tp[:], s[:, k * 128:(k + 1) * 128], tid[:])
                        nc.scalar.copy(sT[:, k, :], tp[:])
                    vp = psA.tile([128, D], F32, tag="Avp")
                    nc.tensor.matmul(vp[:], lhsT=sT[:, 0, :], rhs=tWv[:, 0:D], start=True, stop=False)
                    nc.tensor.matmul(vp[:], lhsT=sT[:, 1, :], rhs=tWv[:, D:2 * D], start=False, stop=False)
                    nc.tensor.matmul(vp[:], lhsT=tones[:], rhs=tbv[:], start=False, stop=True)
                    # vo2[:, h*64:(h+1)*64] = [v_h, v_h] (pair-duplicated)
                    vo2 = pA.tile([128, 2 * D], BF16, tag="Avo2")
                    nc.scalar.copy(_ap(vo2, 0, [[64, 8], [1, 32]]),
                                   _ap(vp, 0, [[32, 8], [1, 32]]))
                    nc.scalar.copy(_ap(vo2, 32, [[64, 8], [1, 32]]),
                                   _ap(vp, 0, [[32, 8], [1, 32]]))
                    # one DMA, all 8 heads: 64-el runs starting (FP+g, 32)
                    dst = bass.AP(vtb.tensor, (FP + i * 128) * (2 * DH) + DH,
                                  [[2 * DH, 128], [STRIPE * 2 * DH, 8], [1, 2 * DH]])
                    srcv = _ap(vo2, 0, [[64, 8], [1, 64]])
                    nc.sync.dma_start(dst, srcv)

                # Pass 2a: value2[:, 0:64] = value_t, split in row blocks
                NBLK = 8
                rows_per = (STRIPE + NBLK - 1) // NBLK
                for bki in range(NBLK):
                    r0 = bki * rows_per
                    n = min(rows_per, STRIPE - r0)
                    dst = bass.AP(v2b.tensor, r0 * CHUNK,
                                  [[STRIPE * CHUNK, 8], [CHUNK, n], [1, 2 * DH]])
                    src = bass.AP(vtb.tensor, r0 * 2 * DH,
                                  [[STRIPE * 2 * DH, 8], [2 * DH, n], [1, 2 * DH]])
                    nc.scalar.dma_start(dst, src)
                # Pass 2b: per level, value2[r, 64:128] = value_t[r+W, :]
                for li, (ls, w) in enumerate(_LEVEL_BOUNDS):
                    le = _LEVEL_BOUNDS[li + 1][0] if li + 1 < len(_LEVEL_BOUNDS) else PAD_LEN + 1 - w
                    r0 = FP + ls - w
                    n = (FP + le) - r0
                    for half in range(2):
                        h0 = half * (n // 2)
                        nh_ = (n // 2) if half == 0 else n - n // 2
                        dst = bass.AP(v2b.tensor, (r0 + h0) * CHUNK + 2 * DH,
                                      [[STRIPE * CHUNK, 8], [CHUNK, nh_], [1, 2 * DH]])
                        src = bass.AP(vtb.tensor, (r0 + h0 + w) * 2 * DH,
                                      [[STRIPE * 2 * DH, 8], [2 * DH, nh_], [1, 2 * DH]])
                        nc.scalar.dma_start(dst, src)

            # ---------------- Phase B: per-query-tile -------------------
            with (
                tc.tile_pool(name="pB", bufs=2) as pB,
                tc.tile_pool(name="pB2", bufs=4) as pB2,
                tc.tile_pool(name="pG", bufs=8) as pG,
                tc.tile_pool(name="pB1", bufs=1) as pB1,
                tc.tile_pool(name="pTw", bufs=3) as pTw,
                tc.tile_pool(name="psB", bufs=2, space="PSUM") as psB,
                tc.tile_pool(name="psB1", bufs=1, space="PSUM") as psB1,
            ):
                def stage1(i):
                    rs = slice(i * 128, (i + 1) * 128)
                    s = pB2.tile([128, D], F32, tag="Bs")
                    nc.sync.dma_start(s[:], srcq[rs, :])
                    p = pB2.tile([128, D], F32, tag="Bp")
                    nc.sync.dma_start(p[:], posq[rs, :])
                    r8 = pB2.tile([128, NL * 2], F32, tag="Br8")
                    nc.sync.dma_start(r8[:], refq[rs, :])

                    q = pB.tile([128, D], F32, tag="Bq")
                    nc.vector.tensor_tensor(out=q[:], in0=s[:], in1=p[:], op=A.add)
                    qT = pB.tile([128, 2, 128], F32, tag="BqT")
                    for k in range(2):
                        tp = psB.tile([128, 128], F32, tag="Btp")
                        nc.tensor.transpose(tp[:], q[:, k * 128:(k + 1) * 128], tid[:])
                        nc.scalar.copy(qT[:, k, :], tp[:])

                    # fused offsets+attention projection: [128, D+NT] psum
                    oap = psB1.tile([128, DOA], F32, tag="Boap")
                    nc.tensor.matmul(oap[:], lhsT=qT[:, 0, :], rhs=tWoa[:, 0:DOA], start=True, stop=False)
                    nc.tensor.matmul(oap[:], lhsT=qT[:, 1, :], rhs=tWoa[:, DOA:2 * DOA], start=False, stop=False)
                    nc.tensor.matmul(oap[:], lhsT=tones[:], rhs=tboa[:], start=False, stop=True)
                    offp = oap[:, 0:D]
                    attp = oap[:, D:DOA]

                    # softmax over the 16 (l,p) per head
                    mx = pB.tile([128, NH], F32, tag="Bmx")
                    nc.vector.tensor_reduce(
                        out=mx[:], in_=_apb(attp, 0, [[16, NH], [1, 16]]),
                        axis=mybir.AxisListType.X, op=A.max)
                    xs = pB1.tile([128, NT], F32, tag="Bxs")
                    nc.vector.tensor_tensor(
                        out=xs[:], in0=attp,
                        in1=_ap(mx, 0, [[1, NH], [0, 16]]), op=A.subtract)
                    es = pB1.tile([128, NT], F32, tag="Bes")
                    nc.scalar.activation(es[:], xs[:], ACTF.Exp)
                    sm = pB.tile([128, NH], F32, tag="Bsm")
                    nc.vector.tensor_reduce(
                        out=sm[:], in_=_ap(es, 0, [[16, NH], [1, 16]]),
                        axis=mybir.AxisListType.X, op=A.add)
                    rcp = pB.tile([128, NH], F32, tag="Brcp")
                    nc.vector.reciprocal(rcp[:], sm[:])
                    aw = pB.tile([128, NT], F32, tag="Baw")
                    nc.vector.tensor_tensor(
                        out=aw[:], in0=es[:],
                        in1=_ap(rcp, 0, [[1, NH], [0, 16]]), op=A.mult)

                    # sampling positions: px = (off - 0.5) + (ref*WH) broadcast
                    rsc = pB.tile([128, NL * 2], F32, tag="Brsc")
                    nc.vector.tensor_tensor(out=rsc[:], in0=r8[:], in1=tdims8[:], op=A.mult)
                    r32 = pB.tile([128, 32], F32, tag="Br32")
                    nc.vector.tensor_copy(out=r32[:], in_=_ap(rsc, 0, [[2, NL], [0, NP], [1, 2]]))
                    px = pB1.tile([128, D], F32, tag="Bpx")
                    nc.vector.scalar_tensor_tensor(
                        out=px[:], in0=offp, scalar=-0.5,
                        in1=_ap(r32, 0, [[0, NH], [1, 32]]), op0=A.add, op1=A.add)

                    # clip to [-1, dim]
                    xt = pB.tile([128, NT], F32, tag="Bxt")
                    nc.vector.scalar_tensor_tensor(
                        out=xt[:], in0=_ap(px, 0, [[2, NT]]), scalar=-1.0,
                        in1=tcW[:], op0=A.max, op1=A.min)
                    yt = pB.tile([128, NT], F32, tag="Byt")
                    nc.vector.scalar_tensor_tensor(
                        out=yt[:], in0=_ap(px, 1, [[2, NT]]), scalar=-1.0,
                        in1=tcH[:], op0=A.max, op1=A.min)

                    # floor + frac (round-to-int via 2^23 trick, then fix up)
                    def floor_frac(src, tagp):
                        r2 = pB.tile([128, NT], F32, tag=tagp + "r2")
                        nc.scalar.activation(r2[:], src[:], ACTF.Identity, bias=t23[:, 0:1])
                        rn = pB.tile([128, NT], F32, tag=tagp + "rn")
                        nc.scalar.activation(rn[:], r2[:], ACTF.Identity, bias=tm23[:, 0:1])
                        fx = pB.tile([128, NT], F32, tag=tagp + "fx")
                        nc.vector.tensor_tensor(out=fx[:], in0=rn[:], in1=src[:], op=A.is_gt)
                        fl = pB.tile([128, NT], F32, tag=tagp + "fl")
                        nc.vector.tensor_tensor(out=fl[:], in0=rn[:], in1=fx[:], op=A.subtract)
                        fr = pB.tile([128, NT], F32, tag=tagp + "fr")
                        nc.vector.tensor_tensor(out=fr[:], in0=src[:], in1=fl[:], op=A.subtract)
                        return fl, fr

                    x0, dx = floor_frac(xt, "Bx")
                    y0, dy = floor_frac(yt, "By")

                    # corner weights with zero-padding masks
                    def corner_w(f0, dfrac, cM1, cM2, tagp):
                        inb1 = pB.tile([128, NT], F32, tag=tagp + "i1")
                        nc.vector.tensor_tensor(out=inb1[:], in0=f0[:], in1=cM1[:], op=A.is_le)
                        m0 = pB.tile([128, NT], F32, tag=tagp + "m0")
                        nc.vector.scalar_tensor_tensor(
                            out=m0[:], in0=f0[:], scalar=0.0, in1=inb1[:],
                            op0=A.is_ge, op1=A.mult)
                        m1 = pB.tile([128, NT], F32, tag=tagp + "m1")
                        nc.vector.tensor_tensor(out=m1[:], in0=f0[:], in1=cM2[:], op=A.is_le)
                        om = pB.tile([128, NT], F32, tag=tagp + "om")
                        nc.scalar.activation(om[:], dfrac[:], ACTF.Identity, bias=tone1[:, 0:1], scale=-1.0)
                        w0 = pB.tile([128, NT], F32, tag=tagp + "w0")
                        nc.vector.tensor_tensor(out=w0[:], in0=om[:], in1=m0[:], op=A.mult)
                        w1 = pB.tile([128, NT], F32, tag=tagp + "w1")
                        nc.vector.tensor_tensor(out=w1[:], in0=dfrac[:], in1=m1[:], op=A.mult)
                        return w0, w1

                    wx0, wx1 = corner_w(x0, dx, tcWm1, tcWm2, "BX")
                    wy0, wy1 = corner_w(y0, dy, tcHm1, tcHm2, "BY")

                    wy0a = pB.tile([128, NT], F32, tag="Bwy0a")
                    nc.vector.tensor_tensor(out=wy0a[:], in0=wy0[:], in1=aw[:], op=A.mult)
                    wy1a = pB.tile([128, NT], F32, tag="Bwy1a")
                    nc.vector.tensor_tensor(out=wy1a[:], in0=wy1[:], in1=aw[:], op=A.mult)

                    # w4[q, 4*triple + corner], bf16, corner order
                    # (y0x0, y0x1, y1x0, y1x1) matching the chunk layout
                    w4 = pB.tile([128, 4 * NT], BF16, tag="Bw4")
                    for jj, (wyj, wxk) in enumerate(
                        [(wy0a, wx0), (wy0a, wx1), (wy1a, wx0), (wy1a, wx1)]
                    ):
                        nc.vector.tensor_tensor(
                            out=_ap(w4, jj, [[4, NT]]), in0=wyj[:], in1=wxk[:], op=A.mult)

                    # chunk offsets: offs = y0*W + x0 + cBASE (y0 UNclipped)
                    t1 = pB.tile([128, NT], F32, tag="Bt1")
                    nc.vector.tensor_tensor(out=t1[:], in0=y0[:], in1=tcW[:], op=A.mult)
                    t2 = pB.tile([128, NT], F32, tag="Bt2")
                    nc.vector.tensor_tensor(out=t2[:], in0=t1[:], in1=x0[:], op=A.add)
                    offs_f = pB.tile([128, NT], F32, tag="Boffsf")
                    nc.vector.tensor_tensor(out=offs_f[:], in0=t2[:], in1=tcBASE[:], op=A.add)

                    # wrapped idx tile: Tw[p16, t*256 + j*8 + qh] =
                    # offs(q = qh*16 + p16, chunk = t*32 + j)
                    oT = pB1.tile([128, 128], F32, tag="BoT")
                    tpo = psB.tile([128, 128], F32, tag="Btp")
                    nc.tensor.transpose(tpo[:], offs_f[:], tid[:])
                    nc.scalar.copy(oT[:], tpo[:])
                    Tw = pTw.tile([128, 4 * 256], I16, tag="BTw")
                    for qh in range(8):
                        tpw = psB1.tile([16, 128], F32, tag="Btpw")
                        nc.tensor.transpose(tpw[:], oT[:, 16 * qh:16 * qh + 16], tid[:])
                        nc.scalar.copy(
                            bass.AP(Tw[:].tensor, Tw[:].offset + qh,
                                    [[list(Tw[:].ap[0])[0], 16], [256, 4], [8, 32]]),
                            _ap(tpw, 0, [[32, 4], [1, 32]]))
                    for rp in range(1, 8):
                        nc.sync.dma_start(Tw[rp * 16:(rp + 1) * 16, :], Tw[0:16, :])

                    gt0 = pG.tile([128, 32, CHUNK], BF16, tag="Bg")
                    gt1 = pG.tile([128, 32, CHUNK], BF16, tag="Bg")
                    gt2 = pG.tile([128, 32, CHUNK], BF16, tag="Bg")
                    gt3 = pG.tile([128, 32, CHUNK], BF16, tag="Bg")
                    gts = [gt0, gt1, gt2, gt3]
                    if "nogather" not in ablate:
                        # one 4096-idx prep per queue; desc-gen rate is the
                        # wall, fewer preps = less fixed overhead
                        for t in range(4):
                            if i > 0:
                                nc.gpsimd.wait_ge(gsems[t], 16 * i)
                            nc.gpsimd.dma_gather(
                                out_ap=gts[t][:],
                                in_ap=value2[2 * t * STRIPE:(2 * t + 2) * STRIPE, :],
                                idxs_ap=Tw[:, t * 256:(t + 1) * 256],
                                num_idxs=NIDX,
                                num_idxs_reg=NIDX, elem_size=CHUNK,
                                single_packet=False, prepare_only=True,
                                sem=gsems[t], queue_num=t)
                            nc.gpsimd.trigger_dma(count=None, queue_num=t)
                    if dbg:
                        nc.sync.dma_start(d_px[rs, :], px[:])
                        nc.sync.dma_start(d_aw[rs, :], aw[:])
                        d_w4f = pB.tile([128, 4 * NT], F32, tag="Bw4f")
                        nc.vector.tensor_copy(out=d_w4f[:], in_=w4[:])
                        nc.sync.dma_start(d_w4[rs, :], d_w4f[:])
                        nc.sync.dma_start(d_ofs[rs, :], offs_f[:])
                    return dict(i=i, s=s, gts=gts, w4=w4)

                def stage2(st):
                    i = st["i"]
                    rs = slice(i * 128, (i + 1) * 128)
                    s = st["s"]
                    gts = st["gts"]
                    w4 = st["w4"]
                    samp = pB.tile([128, D], F32, tag="Bsamp")
                    for t in range(4):
                        g = gts[t]
                        if "nogather" in ablate:
                            nc.vector.memset(g[:, 0, :], 0.0)
                        if "nosamp" in ablate:
                            nc.vector.memset(samp[:, t * 64:(t + 1) * 64], 0.0)
                            continue
                        # explicit completion wait: tile's own DMASW dep
                        # wiring under-synced multi-queue preps (proven in
                        # CoreSim); the descriptor sem is ground truth.
                        if "nogather" not in ablate:
                            nc.vector.wait_ge(gsems[t], 16 * (i + 1))
                        # g *= w4 broadcast (in place)
                        sw = g
                        nc.vector.tensor_tensor(
                            out=_ap(g, 0, [[128, 32], [32, 4], [1, 32]]),
                            in0=_ap(g, 0, [[128, 32], [32, 4], [1, 32]]),
                            in1=_ap(w4, t * 128, [[4, 32], [1, 4], [0, 32]]),
                            op=A.mult)
                        for n in (64, 32, 16, 8, 4):
                            nc.vector.tensor_tensor(
                                out=_ap(sw, 0, [[32, n], [1, 32]]),
                                in0=_ap(sw, 0, [[64, n], [1, 32]]),
                                in1=_ap(sw, 32, [[64, n], [1, 32]]), op=A.add)
                        nc.vector.tensor_tensor(
                            out=_ap(samp, t * 64, [[32, 2], [1, 32]]),
                            in0=_ap(sw, 0, [[64, 2], [1, 32]]),
                            in1=_ap(sw, 32, [[64, 2], [1, 32]]), op=A.add)

                    # output projection
                    sT = pB.tile([128, 2, 128], F32, tag="BsT")
                    for k in range(2):
                        tp = psB.tile([128, 128], F32, tag="Btp")
                        nc.tensor.transpose(tp[:], samp[:, k * 128:(k + 1) * 128], tid[:])
                        nc.scalar.copy(sT[:, k, :], tp[:])
                    o2p = psB.tile([128, D], F32, tag="Bmm")
                    nc.tensor.matmul(o2p[:], lhsT=sT[:, 0, :], rhs=tWout[:, 0:D], start=True, stop=False)
                    nc.tensor.matmul(o2p[:], lhsT=sT[:, 1, :], rhs=tWout[:, D:2 * D], start=False, stop=False)
                    nc.tensor.matmul(o2p[:], lhsT=tones[:], rhs=tbout[:], start=False, stop=True)

                    # residual + layernorm
                    def layer_norm(inp_sbuf, res_psum, gt, bt, tagp):
                        x1 = pB.tile([128, D], F32, tag=tagp + "x1")
                        sums = pB.tile([128, 1], F32, tag=tagp + "su")
                        nc.vector.scalar_tensor_tensor(
                            out=x1[:], in0=inp_sbuf[:], scalar=0.0, in1=res_psum[:],
                            op0=A.add, op1=A.add, accum_out=sums[:])
                        negm = pB.tile([128, 1], F32, tag=tagp + "nm")
                        nc.scalar.mul(negm[:], sums[:], -1.0 / D)
                        sq = pB.tile([128, D], F32, tag=tagp + "sq")
                        ssq = pB.tile([128, 1], F32, tag=tagp + "ss")
                        nc.scalar.activation(sq[:], x1[:], ACTF.Square,
                                             bias=negm[:, 0:1], accum_out=ssq[:])
                        sd = pB.tile([128, 1], F32, tag=tagp + "sd")
                        nc.scalar.activation(sd[:], ssq[:], ACTF.Sqrt,
                                             scale=1.0 / D, bias=teps[:, 0:1])
                        rstd = pB.tile([128, 1], F32, tag=tagp + "rs")
                        nc.vector.reciprocal(rstd[:], sd[:])
                        xh = pB.tile([128, D], F32, tag=tagp + "xh")
                        nc.vector.tensor_scalar(
                            out=xh[:], in0=x1[:], scalar1=negm[:, 0:1],
                            scalar2=rstd[:, 0:1], op0=A.add, op1=A.mult)
                        yv = pB.tile([128, D], F32, tag=tagp + "y")
                        nc.vector.tensor_tensor(out=yv[:], in0=xh[:], in1=gt[:], op=A.mult)
                        nc.vector.tensor_tensor(out=yv[:], in0=yv[:], in1=bt[:], op=A.add)
                        return yv

                    y1v = layer_norm(s, o2p, tg1, tbe1, "BL1")

                    # FFN
                    yT = pB.tile([128, 2, 128], F32, tag="ByT")
                    for k in range(2):
                        tp = psB.tile([128, 128], F32, tag="Btp")
                        nc.tensor.transpose(tp[:], y1v[:, k * 128:(k + 1) * 128], tid[:])
                        nc.scalar.copy(yT[:, k, :], tp[:])
                    h1 = pB1.tile([128, DFF], F32, tag="Bh1")
                    for j in range(8):
                        js = slice(j * 128, (j + 1) * 128)
                        hp = psB.tile([128, 128], F32, tag="Bhp")
                        nc.tensor.matmul(hp[:], lhsT=tW1[:, 0 * DFF + j * 128:0 * DFF + (j + 1) * 128],
                                         rhs=yT[:, 0, :], start=True, stop=False)
                        nc.tensor.matmul(hp[:], lhsT=tW1[:, 1 * DFF + j * 128:1 * DFF + (j + 1) * 128],
                                         rhs=yT[:, 1, :], start=False, stop=True)
                        nc.scalar.activation(h1[:, js], hp[:], ACTF.Relu,
                                             bias=tb1T[:, j:j + 1])
                    o3p = psB.tile([128, D], F32, tag="Bmm")
                    for j in range(8):
                        js = slice(j * 128, (j + 1) * 128)
                        nc.tensor.matmul(o3p[:], lhsT=h1[:, js], rhs=tW2[:, j * D:(j + 1) * D],
                                         start=(j == 0), stop=False)
                    nc.tensor.matmul(o3p[:], lhsT=tones[:], rhs=tb2[:], start=False, stop=True)

                    y2v = layer_norm(y1v, o3p, tg2, tbe2, "BL2")
                    nc.sync.dma_start(outq[rs, :], y2v[:])
                    if dbg:
                        nc.sync.dma_start(d_samp[rs, :], samp[:])

                # software pipeline: tile i+1's index/prep front-end is
                # emitted before tile i's consumers so DVE's gather waits
                # never starve GpSimd desc-gen of ready idx tables
                prev = None
                for i in range(0 if "nob" in ablate else N_Q_TILES):
                    cur = stage1(i)
                    if prev is not None:
                        stage2(prev)
                    prev = cur
                if prev is not None:
                    stage2(prev)

    nc.compile()
    return nc


# ----------------------------------------------------------------------
# host-side wrapper
# ----------------------------------------------------------------------
_NC_CACHE = None


def _get_nc():
    global _NC_CACHE
    if _NC_CACHE is None:
        _NC_CACHE = build()
    return _NC_CACHE


def make_consts():
    h_i, l_i, p_i = np.meshgrid(np.arange(NH), np.arange(NL), np.arange(NP), indexing="ij")
    Wl = np.array([w for (_, w) in SPATIAL], np.float32)
    Hl = np.array([h for (h, _) in SPATIAL], np.float32)
    lw = Wl[l_i].reshape(-1)
    lh = Hl[l_i].reshape(-1)
    base = ((h_i % 2) * STRIPE + FP
            + np.array(LEVEL_START, np.float32)[l_i] + 1).reshape(-1)
    rep = lambda v: np.tile(v[None, :].astype(np.float32), (128, 1))
    dims8 = np.zeros(NL * 2, np.float32)
    dims8[0::2] = Wl
    dims8[1::2] = Hl
    return {
        "cW": rep(lw), "cH": rep(lh),
        "cWm1": rep(lw - 1), "cHm1": rep(lh - 1),
        "cWm2": rep(lw - 2), "cHm2": rep(lh - 2),
        "cBASE": rep(base),
        "dims8": rep(dims8),
        "ident": np.eye(128, dtype=np.float32),
        "ones_row": np.ones((1, 128), np.float32),
    }


SHARD_STARTS = [0, 3324, 6648, 9972]
SHARD_SIZES = [3324, 3324, 3324, 3322]

_V2_ZEROS = None
_VT_ZEROS = None


def _v2_zeros():
    global _V2_ZEROS
    if _V2_ZEROS is None:
        _V2_ZEROS = np.zeros((NH * STRIPE, CHUNK), ml_dtypes.bfloat16)
    return _V2_ZEROS


def _vt_zeros():
    global _VT_ZEROS
    if _VT_ZEROS is None:
        _VT_ZEROS = np.zeros((NH * STRIPE, 2 * DH), ml_dtypes.bfloat16)
    return _VT_ZEROS


def make_in_maps(inputs):
    consts = make_consts()
    in_maps = []
    for core in range(8):
        b, c = core // 4, core % 4
        st, sz = SHARD_STARTS[c], SHARD_SIZES[c]
        src_full = np.zeros((PAD_LEN, D), np.float32)
        src_full[:LEN] = inputs["src"][b]
        srcq = np.zeros((Q_SH, D), np.float32)
        srcq[:sz] = inputs["src"][b, st:st + sz]
        posq = np.zeros((Q_SH, D), np.float32)
        posq[:sz] = inputs["pos"][b, st:st + sz]
        refq = np.full((Q_SH, NL * 2), 0.5, np.float32)
        refq[:sz] = inputs["reference_points"][b, st:st + sz].reshape(sz, NL * 2)
        m = {
            "src_full": src_full, "srcq": srcq, "posq": posq, "refq": refq,
            "Wv": inputs["W_value"],
            "Woa": np.concatenate([inputs["W_off"], inputs["W_attn"]], axis=1),
            "Wout": inputs["W_out"],
            "W1": inputs["W1"], "W2": inputs["W2"],
            "bv": inputs["b_value"][None, :],
            "boa": np.concatenate([inputs["b_off"], inputs["b_attn"]])[None, :],
            "bout": inputs["b_out"][None, :],
            "b1T": np.asarray(inputs["b1"]).reshape(DFF // 128, 128).T,
            "b2": inputs["b2"][None, :],
            "g1r": np.tile(inputs["g1"][None, :], (128, 1)),
            "be1r": np.tile(inputs["be1"][None, :], (128, 1)),
            "g2r": np.tile(inputs["g2"][None, :], (128, 1)),
            "be2r": np.tile(inputs["be2"][None, :], (128, 1)),
        }
        for k in ("cW", "cH", "cWm1", "cHm1", "cWm2", "cHm2", "cBASE", "dims8", "ident", "ones_row"):
            m[k] = consts[k]
        m = {k: np.ascontiguousarray(v, np.float32) for k, v in m.items()}
        m["value2"] = _v2_zeros()
        m["value_t"] = _vt_zeros()
        in_maps.append(m)
    return in_maps


def assemble_out(results):
    out = np.empty((2, LEN, D), np.float32)
    for core in range(8):
        b, c = core // 4, core % 4
        st, sz = SHARD_STARTS[c], SHARD_SIZES[c]
        out[b, st:st + sz] = results[core]["outq"][:sz]
    return out


def run(inputs, trace=False, **kw):
    nc = _get_nc()
    in_maps = make_in_maps(inputs)
    res = run_bass_kernel_spmd(nc, in_maps, core_ids=list(range(8)), trace=trace, **kw)
    return assemble_out(res.results), res


def kernel(**inputs):
    out, _ = run(inputs)
    return out



# revision 31
# speedup vs baseline: 1.0617x; 1.0617x over previous
"""Deformable-DETR transformer encoder layer on 8 Trainium2 NeuronCores.

Sharding: data-parallel over batch (B=2 -> 4 cores per batch element),
sequence-parallel over queries within the batch group. Each core computes
the full value memory for its batch element, stores it to DRAM in a
4-corner "pair" layout (bf16, 128 elems = 256B per chunk: both x corners
of both y rows of a bilinear sample), then gathers one chunk per
(head, level, point) via SWDGE dma_gather preps spread over 4 queues.

Self-contained: hardcodes all shapes/constants from the problem spec.
"""

import numpy as np
import ml_dtypes

import concourse.bass as bass
import concourse.mybir as mybir
import concourse.tile as tile
from concourse import bacc
from concourse.bass_utils import run_bass_kernel_spmd

F32 = mybir.dt.float32
I32 = mybir.dt.int32
I16 = mybir.dt.int16
BF16 = mybir.dt.bfloat16

# ---- problem constants -------------------------------------------------
SPATIAL = [(100, 100), (50, 50), (25, 25), (13, 13)]
LEVEL_START = [0, 10000, 12500, 13125]
LEN = 13294
D = 256
NH = 8
NL = 4
NP = 4
DH = 32
DFF = 1024
EPS = 1e-5

PAD_LEN = 13312           # 104 * 128, full-sequence padded length
N_FULL_TILES = PAD_LEN // 128
Q_SH = 3328               # 26 * 128, per-core query shard (padded)
N_Q_TILES = Q_SH // 128

FP = 104                  # front pad rows per head stripe (>= max W)
STRIPE = FP + PAD_LEN + 1  # 13417 rows per head stripe
CHUNK = 4 * DH            # 128 elems per chunk: (y0,y1) x (x0,x1) x DH
NT = NH * NL * NP         # 128 (h,l,p) triples = chunks per query
NIDX = 32 * 128           # idxs per gather (2 heads x 16 lp x 128 queries)

TWO23 = float(3 << 22)  # 1.5*2^23 magic round constant

# level-of-row segments for a 128-row tile: (start, n, W)
_LEVEL_BOUNDS = [(0, 100), (10000, 50), (12500, 25), (13125, 13)]


def _tile_segs(i):
    g0, g1 = i * 128, (i + 1) * 128
    segs = []
    for k, (ls, w) in enumerate(_LEVEL_BOUNDS):
        le = _LEVEL_BOUNDS[k + 1][0] if k + 1 < len(_LEVEL_BOUNDS) else PAD_LEN
        s, e = max(g0, ls), min(g1, le)
        if s < e:
            segs.append((s - g0, e - s, w))
    return segs


def _ap(t, offset_elems, dims):
    """Custom free-dim AP view of an SBUF tile (keeps full partition dim)."""
    base = t[:]
    return bass.AP(base.tensor, base.offset + offset_elems, [list(base.ap[0])] + [list(d) for d in dims])


def _apb(base, offset_elems, dims):
    """Custom free-dim AP from a pre-sliced base AP (partition sub-range)."""
    return bass.AP(base.tensor, base.offset + offset_elems, [list(base.ap[0])] + [list(d) for d in dims])


def build(dbg=False, ablate=()):
    nc = bacc.Bacc("TRN2", target_bir_lowering=False, debug=False, num_devices=8,
                   num_swdge_queues=4, dynamic_dma_scratch_size=40960)
    A = mybir.AluOpType
    ACTF = mybir.ActivationFunctionType

    def param(name, shape, dtype=F32, out=False):
        return nc.declare_dram_parameter(name, list(shape), dtype, isOutput=out)

    src_full = param("src_full", [PAD_LEN, D])
    srcq = param("srcq", [Q_SH, D])
    posq = param("posq", [Q_SH, D])
    refq = param("refq", [Q_SH, NL * 2])
    value2 = param("value2", [NH * STRIPE, CHUNK], BF16)
    Wv = param("Wv", [D, D], BF16)
    Woa = param("Woa", [D, D + NT])
    Wout = param("Wout", [D, D])
    W1 = param("W1", [D, DFF])
    W2 = param("W2", [DFF, D])
    bv = param("bv", [1, D], BF16)
    boa = param("boa", [1, D + NT])
    bout = param("bout", [1, D])
    b1T = param("b1T", [128, DFF // 128])
    b2 = param("b2", [1, D])
    g1r = param("g1r", [128, D])
    be1r = param("be1r", [128, D])
    g2r = param("g2r", [128, D])
    be2r = param("be2r", [128, D])
    ident = param("ident", [128, 128])
    ones_row = param("ones_row", [1, 128])
    cW = param("cW", [128, NT])
    cH = param("cH", [128, NT])
    cWm1 = param("cWm1", [128, NT])
    cHm1 = param("cHm1", [128, NT])
    cWm2 = param("cWm2", [128, NT])
    cHm2 = param("cHm2", [128, NT])
    cBASE = param("cBASE", [128, NT])
    dims8 = param("dims8", [128, NL * 2])
    outq = param("outq", [Q_SH, D], out=True)
    if dbg:
        d_px = param("d_px", [Q_SH, D], out=True)
        d_aw = param("d_aw", [Q_SH, NT], out=True)
        d_w4 = param("d_w4", [Q_SH, 4 * NT], out=True)
        d_ofs = param("d_ofs", [Q_SH, NT], out=True)
        d_samp = param("d_samp", [Q_SH, D], out=True)

    with tile.TileContext(nc) as tc:
        gsems = [nc.alloc_semaphore(f"gsem{t}") for t in range(4)]
        with (
            tc.tile_pool(name="const", bufs=1) as cp,
        ):
            v2b = value2[:]

            def cload(src_ap, p, n, tag):
                t = cp.tile([p, n], F32, tag=tag)
                nc.sync.dma_start(t[:], src_ap[:])
                return t

            tWv = cp.tile([128, 2 * D], BF16, tag="tWv")
            nc.sync.dma_start(tWv[:, 0:D], Wv[0:128, :])
            nc.sync.dma_start(tWv[:, D:2 * D], Wv[128:256, :])
            DOA = D + NT
            tWoa = cp.tile([128, 2 * DOA], F32, tag="tWoa")
            nc.sync.dma_start(tWoa[:, 0:DOA], Woa[0:128, :])
            nc.sync.dma_start(tWoa[:, DOA:2 * DOA], Woa[128:256, :])
            tWout = cp.tile([128, 2 * D], F32, tag="tWout")
            nc.sync.dma_start(tWout[:, 0:D], Wout[0:128, :])
            nc.sync.dma_start(tWout[:, D:2 * D], Wout[128:256, :])
            tW1 = cp.tile([128, 2 * DFF], F32, tag="tW1")
            nc.sync.dma_start(tW1[:, 0:DFF], W1[0:128, :])
            nc.sync.dma_start(tW1[:, DFF:2 * DFF], W1[128:256, :])
            tW2 = cp.tile([128, 8 * D], F32, tag="tW2")
            for j in range(8):
                nc.sync.dma_start(tW2[:, j * D:(j + 1) * D], W2[j * 128:(j + 1) * 128, :])

            tbv = cp.tile([1, D], BF16, tag="tbv")
            nc.sync.dma_start(tbv[:], bv[:])
            tboa = cload(boa, 1, DOA, "tboa")
            tbout = cload(bout, 1, D, "tbout")
            tb1T = cload(b1T, 128, DFF // 128, "tb1T")
            tb2 = cload(b2, 1, D, "tb2")
            tg1 = cload(g1r, 128, D, "tg1")
            tbe1 = cload(be1r, 128, D, "tbe1")
            tg2 = cload(g2r, 128, D, "tg2")
            tbe2 = cload(be2r, 128, D, "tbe2")
            tid = cload(ident, 128, 128, "tid")
            tones = cload(ones_row, 1, 128, "tones")
            tones_bf = cp.tile([1, 128], BF16, tag="tones_bf")
            nc.vector.tensor_copy(out=tones_bf[:], in_=tones[:])
            tcW = cload(cW, 128, NT, "tcW")
            tcH = cload(cH, 128, NT, "tcH")
            tcWm1 = cload(cWm1, 128, NT, "tcWm1")
            tcHm1 = cload(cHm1, 128, NT, "tcHm1")
            tcWm2 = cload(cWm2, 128, NT, "tcWm2")
            tcHm2 = cload(cHm2, 128, NT, "tcHm2")
            tcBASE = cload(cBASE, 128, NT, "tcBASE")
            tdims8 = cload(dims8, 128, NL * 2, "tdims8")

            # small scalar constants for ACT bias operands
            def cconst(val, tag):
                t = cp.tile([128, 1], F32, tag=tag)
                nc.vector.memset(t[:], val)
                return t

            t23 = cconst(TWO23, "t23")
            tm23 = cconst(-TWO23, "tm23")
            tone1 = cconst(1.0, "tone1")
            teps = cconst(EPS, "teps")

            # ---------------- Phase A: value projection ----------------
            # Direct build of the 4-corner pair table: per 128-row tile of
            # v = src @ Wv (bf16 matmuls), write the pair-duplicated rows
            # straight into both halves of value2 with 64-element runs that
            # straddle row boundaries (row width 128: a 64-run starting at
            # col 32 lands [r,32:64]+[r+1,0:32]; at col 96 it lands
            # [r,96:128]+[r+1,64:96]). No value_t intermediate, no
            # DRAM->DRAM pass: v2done is pass-1-end, ~2.5x sooner, which
            # directly advances the first gather prep (it wants all of
            # value2 written).
            with (
                tc.tile_pool(name="pA", bufs=4) as pA,
                tc.tile_pool(name="psA", bufs=4, space="PSUM") as psA,
            ):
                for i in range(0 if "noa" in ablate else N_FULL_TILES):
                    rs = slice(i * 128, (i + 1) * 128)
                    s = pA.tile([128, D], F32, tag="As")
                    nc.sync.dma_start(s[:], src_full[rs, :])
                    sT = pA.tile([128, 2, 128], BF16, tag="AsT")
                    for k in range(2):
                        tp = psA.tile([128, 128], F32, tag="Atp")
                        nc.tensor.transpose(tp[:], s[:, k * 128:(k + 1) * 128], tid[:])
                        nc.vector.tensor_copy(out=sT[:, k, :], in_=tp[:])
                    vp = psA.tile([128, D], F32, tag="Avp")
                    nc.tensor.matmul(vp[:], lhsT=sT[:, 0, :], rhs=tWv[:, 0:D], start=True, stop=False)
                    nc.tensor.matmul(vp[:], lhsT=sT[:, 1, :], rhs=tWv[:, D:2 * D], start=False, stop=False)
                    nc.tensor.matmul(vp[:], lhsT=tones_bf[:], rhs=tbv[:], start=False, stop=True)
                    # vo2[:, h*64:(h+1)*64] = [v_h, v_h] (pair-duplicated)
                    vo2 = pA.tile([128, 2 * D], BF16, tag="Avo2")
                    nc.vector.tensor_copy(out=_ap(vo2, 0, [[64, 8], [1, 32]]),
                                          in_=_ap(vp, 0, [[32, 8], [1, 32]]))
                    nc.vector.tensor_copy(out=_ap(vo2, 32, [[64, 8], [1, 32]]),
                                          in_=_ap(vp, 0, [[32, 8], [1, 32]]))
                    # y0 half: value2[FP+g, 32:64] = v_g, [FP+g+1, 0:32] = v_g
                    dst = bass.AP(v2b.tensor, (FP + i * 128) * CHUNK + DH,
                                  [[CHUNK, 128], [STRIPE * CHUNK, 8], [1, 2 * DH]])
                    nc.scalar.dma_start(dst, _ap(vo2, 0, [[64, 8], [1, 64]]))
                    # y1 half, per level segment: two 32-el-run writes (a
                    # 64-run would wrap to col 0 of the next row, not 64):
                    # value2[FP+g-W, 96:128] = v_g and [FP+g-W+1, 64:96] = v_g
                    for (r_off, n, w) in _tile_segs(i):
                        r0 = FP + i * 128 + r_off - w
                        for (rr, cc) in ((r0, 3 * DH), (r0 + 1, 2 * DH)):
                            dstb = bass.AP(v2b.tensor, rr * CHUNK + cc,
                                           [[CHUNK, n], [STRIPE * CHUNK, 8], [1, DH]])
                            srcb = _apb(vo2[r_off:r_off + n, :], 0,
                                        [[64, 8], [1, DH]])
                            nc.scalar.dma_start(dstb, srcb)

            # ---------------- Phase B: per-query-tile -------------------
            with (
                tc.tile_pool(name="pB", bufs=2) as pB,
                tc.tile_pool(name="pB2", bufs=4) as pB2,
                tc.tile_pool(name="pG", bufs=8) as pG,
                tc.tile_pool(name="pB1", bufs=1) as pB1,
                tc.tile_pool(name="pTw", bufs=3) as pTw,
                tc.tile_pool(name="psB", bufs=2, space="PSUM") as psB,
                tc.tile_pool(name="psB1", bufs=1, space="PSUM") as psB1,
            ):
                def stage1(i):
                    rs = slice(i * 128, (i + 1) * 128)
                    s = pB2.tile([128, D], F32, tag="Bs")
                    nc.sync.dma_start(s[:], srcq[rs, :])
                    p = pB2.tile([128, D], F32, tag="Bp")
                    nc.sync.dma_start(p[:], posq[rs, :])
                    r8 = pB2.tile([128, NL * 2], F32, tag="Br8")
                    nc.sync.dma_start(r8[:], refq[rs, :])

                    q = pB.tile([128, D], F32, tag="Bq")
                    nc.vector.tensor_tensor(out=q[:], in0=s[:], in1=p[:], op=A.add)
                    qT = pB.tile([128, 2, 128], F32, tag="BqT")
                    for k in range(2):
                        tp = psB.tile([128, 128], F32, tag="Btp")
                        nc.tensor.transpose(tp[:], q[:, k * 128:(k + 1) * 128], tid[:])
                        nc.scalar.copy(qT[:, k, :], tp[:])

                    # fused offsets+attention projection: [128, D+NT] psum
                    oap = psB1.tile([128, DOA], F32, tag="Boap")
                    nc.tensor.matmul(oap[:], lhsT=qT[:, 0, :], rhs=tWoa[:, 0:DOA], start=True, stop=False)
                    nc.tensor.matmul(oap[:], lhsT=qT[:, 1, :], rhs=tWoa[:, DOA:2 * DOA], start=False, stop=False)
                    nc.tensor.matmul(oap[:], lhsT=tones[:], rhs=tboa[:], start=False, stop=True)
                    offp = oap[:, 0:D]
                    attp = oap[:, D:DOA]

                    # softmax over the 16 (l,p) per head
                    mx = pB.tile([128, NH], F32, tag="Bmx")
                    nc.vector.tensor_reduce(
                        out=mx[:], in_=_apb(attp, 0, [[16, NH], [1, 16]]),
                        axis=mybir.AxisListType.X, op=A.max)
                    xs = pB1.tile([128, NT], F32, tag="Bxs")
                    nc.vector.tensor_tensor(
                        out=xs[:], in0=attp,
                        in1=_ap(mx, 0, [[1, NH], [0, 16]]), op=A.subtract)
                    es = pB1.tile([128, NT], F32, tag="Bes")
                    nc.scalar.activation(es[:], xs[:], ACTF.Exp)
                    sm = pB.tile([128, NH], F32, tag="Bsm")
                    nc.vector.tensor_reduce(
                        out=sm[:], in_=_ap(es, 0, [[16, NH], [1, 16]]),
                        axis=mybir.AxisListType.X, op=A.add)
                    rcp = pB.tile([128, NH], F32, tag="Brcp")
                    nc.vector.reciprocal(rcp[:], sm[:])
                    aw = pB.tile([128, NT], F32, tag="Baw")
                    nc.vector.tensor_tensor(
                        out=aw[:], in0=es[:],
                        in1=_ap(rcp, 0, [[1, NH], [0, 16]]), op=A.mult)

                    # sampling positions: px = (off - 0.5) + (ref*WH) broadcast
                    rsc = pB.tile([128, NL * 2], F32, tag="Brsc")
                    nc.vector.tensor_tensor(out=rsc[:], in0=r8[:], in1=tdims8[:], op=A.mult)
                    r32 = pB.tile([128, 32], F32, tag="Br32")
                    nc.vector.tensor_copy(out=r32[:], in_=_ap(rsc, 0, [[2, NL], [0, NP], [1, 2]]))
                    px = pB1.tile([128, D], F32, tag="Bpx")
                    nc.vector.scalar_tensor_tensor(
                        out=px[:], in0=offp, scalar=-0.5,
                        in1=_ap(r32, 0, [[0, NH], [1, 32]]), op0=A.add, op1=A.add)

                    # clip to [-1, dim]
                    xt = pB.tile([128, NT], F32, tag="Bxt")
                    nc.vector.scalar_tensor_tensor(
                        out=xt[:], in0=_ap(px, 0, [[2, NT]]), scalar=-1.0,
                        in1=tcW[:], op0=A.max, op1=A.min)
                    yt = pB.tile([128, NT], F32, tag="Byt")
                    nc.vector.scalar_tensor_tensor(
                        out=yt[:], in0=_ap(px, 1, [[2, NT]]), scalar=-1.0,
                        in1=tcH[:], op0=A.max, op1=A.min)

                    # floor + frac (round-to-int via 2^23 trick, then fix up)
                    def floor_frac(src, tagp):
                        r2 = pB.tile([128, NT], F32, tag=tagp + "r2")
                        nc.scalar.activation(r2[:], src[:], ACTF.Identity, bias=t23[:, 0:1])
                        rn = pB.tile([128, NT], F32, tag=tagp + "rn")
                        nc.scalar.activation(rn[:], r2[:], ACTF.Identity, bias=tm23[:, 0:1])
                        fx = pB.tile([128, NT], F32, tag=tagp + "fx")
                        nc.vector.tensor_tensor(out=fx[:], in0=rn[:], in1=src[:], op=A.is_gt)
                        fl = pB.tile([128, NT], F32, tag=tagp + "fl")
                        nc.vector.tensor_tensor(out=fl[:], in0=rn[:], in1=fx[:], op=A.subtract)
                        fr = pB.tile([128, NT], F32, tag=tagp + "fr")
                        nc.vector.tensor_tensor(out=fr[:], in0=src[:], in1=fl[:], op=A.subtract)
                        return fl, fr

                    x0, dx = floor_frac(xt, "Bx")
                    y0, dy = floor_frac(yt, "By")

                    # corner weights with zero-padding masks
                    def corner_w(f0, dfrac, cM1, cM2, tagp):
                        inb1 = pB.tile([128, NT], F32, tag=tagp + "i1")
                        nc.vector.tensor_tensor(out=inb1[:], in0=f0[:], in1=cM1[:], op=A.is_le)
                        m0 = pB.tile([128, NT], F32, tag=tagp + "m0")
                        nc.vector.scalar_tensor_tensor(
                            out=m0[:], in0=f0[:], scalar=0.0, in1=inb1[:],
                            op0=A.is_ge, op1=A.mult)
                        m1 = pB.tile([128, NT], F32, tag=tagp + "m1")
                        nc.vector.tensor_tensor(out=m1[:], in0=f0[:], in1=cM2[:], op=A.is_le)
                        om = pB.tile([128, NT], F32, tag=tagp + "om")
                        nc.scalar.activation(om[:], dfrac[:], ACTF.Identity, bias=tone1[:, 0:1], scale=-1.0)
                        w0 = pB.tile([128, NT], F32, tag=tagp + "w0")
                        nc.vector.tensor_tensor(out=w0[:], in0=om[:], in1=m0[:], op=A.mult)
                        w1 = pB.tile([128, NT], F32, tag=tagp + "w1")
                        nc.vector.tensor_tensor(out=w1[:], in0=dfrac[:], in1=m1[:], op=A.mult)
                        return w0, w1

                    wx0, wx1 = corner_w(x0, dx, tcWm1, tcWm2, "BX")
                    wy0, wy1 = corner_w(y0, dy, tcHm1, tcHm2, "BY")

                    wy0a = pB.tile([128, NT], F32, tag="Bwy0a")
                    nc.vector.tensor_tensor(out=wy0a[:], in0=wy0[:], in1=aw[:], op=A.mult)
                    wy1a = pB.tile([128, NT], F32, tag="Bwy1a")
                    nc.vector.tensor_tensor(out=wy1a[:], in0=wy1[:], in1=aw[:], op=A.mult)

                    # w4[q, 4*triple + corner], bf16, corner order
                    # (y0x0, y0x1, y1x0, y1x1) matching the chunk layout
                    w4 = pB.tile([128, 4 * NT], BF16, tag="Bw4")
                    for jj, (wyj, wxk) in enumerate(
                        [(wy0a, wx0), (wy0a, wx1), (wy1a, wx0), (wy1a, wx1)]
                    ):
                        nc.vector.tensor_tensor(
                            out=_ap(w4, jj, [[4, NT]]), in0=wyj[:], in1=wxk[:], op=A.mult)

                    # chunk offsets: offs = y0*W + x0 + cBASE (y0 UNclipped)
                    t1 = pB.tile([128, NT], F32, tag="Bt1")
                    nc.vector.tensor_tensor(out=t1[:], in0=y0[:], in1=tcW[:], op=A.mult)
                    t2 = pB.tile([128, NT], F32, tag="Bt2")
                    nc.vector.tensor_tensor(out=t2[:], in0=t1[:], in1=x0[:], op=A.add)
                    offs_f = pB.tile([128, NT], F32, tag="Boffsf")
                    nc.vector.tensor_tensor(out=offs_f[:], in0=t2[:], in1=tcBASE[:], op=A.add)

                    # wrapped idx tile: Tw[p16, t*256 + j*8 + qh] =
                    # offs(q = qh*16 + p16, chunk = t*32 + j)
                    oT = pB1.tile([128, 128], F32, tag="BoT")
                    tpo = psB.tile([128, 128], F32, tag="Btp")
                    nc.tensor.transpose(tpo[:], offs_f[:], tid[:])
                    nc.scalar.copy(oT[:], tpo[:])
                    Tw = pTw.tile([128, 4 * 256], I16, tag="BTw")
                    for qh in range(8):
                        tpw = psB1.tile([16, 128], F32, tag="Btpw")
                        nc.tensor.transpose(tpw[:], oT[:, 16 * qh:16 * qh + 16], tid[:])
                        nc.scalar.copy(
                            bass.AP(Tw[:].tensor, Tw[:].offset + qh,
                                    [[list(Tw[:].ap[0])[0], 16], [256, 4], [8, 32]]),
                            _ap(tpw, 0, [[32, 4], [1, 32]]))
                    for rp in range(1, 8):
                        nc.sync.dma_start(Tw[rp * 16:(rp + 1) * 16, :], Tw[0:16, :])

                    gt0 = pG.tile([128, 32, CHUNK], BF16, tag="Bg")
                    gt1 = pG.tile([128, 32, CHUNK], BF16, tag="Bg")
                    gt2 = pG.tile([128, 32, CHUNK], BF16, tag="Bg")
                    gt3 = pG.tile([128, 32, CHUNK], BF16, tag="Bg")
                    gts = [gt0, gt1, gt2, gt3]
                    if "nogather" not in ablate:
                        # one 4096-idx prep per queue; desc-gen rate is the
                        # wall, fewer preps = less fixed overhead
                        for t in range(4):
                            if i > 1:
                                # gt pool is 2 tiles deep: slot i%2 is free
                                # once gather i-2 completed (the tighter
                                # 16*i wait cost ~7us/tile of engine idle)
                                nc.gpsimd.wait_ge(gsems[t], 16 * (i - 1))
                            nc.gpsimd.dma_gather(
                                out_ap=gts[t][:],
                                in_ap=value2[2 * t * STRIPE:(2 * t + 2) * STRIPE, :],
                                idxs_ap=Tw[:, t * 256:(t + 1) * 256],
                                num_idxs=NIDX,
                                num_idxs_reg=NIDX, elem_size=CHUNK,
                                single_packet=False, prepare_only=True,
                                sem=gsems[t], queue_num=t)
                            nc.gpsimd.trigger_dma(count=None, queue_num=t)
                    if dbg:
                        nc.sync.dma_start(d_px[rs, :], px[:])
                        nc.sync.dma_start(d_aw[rs, :], aw[:])
                        d_w4f = pB.tile([128, 4 * NT], F32, tag="Bw4f")
                        nc.vector.tensor_copy(out=d_w4f[:], in_=w4[:])
                        nc.sync.dma_start(d_w4[rs, :], d_w4f[:])
                        nc.sync.dma_start(d_ofs[rs, :], offs_f[:])
                    return dict(i=i, s=s, gts=gts, w4=w4)

                def stage2(st):
                    i = st["i"]
                    rs = slice(i * 128, (i + 1) * 128)
                    s = st["s"]
                    gts = st["gts"]
                    w4 = st["w4"]
                    samp = pB.tile([128, D], F32, tag="Bsamp")
                    for t in range(4):
                        g = gts[t]
                        if "nogather" in ablate:
                            nc.vector.memset(g[:, 0, :], 0.0)
                        if "nosamp" in ablate:
                            nc.vector.memset(samp[:, t * 64:(t + 1) * 64], 0.0)
                            continue
                        # explicit completion wait: tile's own DMASW dep
                        # wiring under-synced multi-queue preps (proven in
                        # CoreSim); the descriptor sem is ground truth.
                        if "nogather" not in ablate:
                            nc.vector.wait_ge(gsems[t], 16 * (i + 1))
                        # g *= w4 broadcast (in place)
                        sw = g
                        nc.vector.tensor_tensor(
                            out=_ap(g, 0, [[128, 32], [32, 4], [1, 32]]),
                            in0=_ap(g, 0, [[128, 32], [32, 4], [1, 32]]),
                            in1=_ap(w4, t * 128, [[4, 32], [1, 4], [0, 32]]),
                            op=A.mult)
                        for n in (64, 32, 16, 8, 4):
                            nc.vector.tensor_tensor(
                                out=_ap(sw, 0, [[32, n], [1, 32]]),
                                in0=_ap(sw, 0, [[64, n], [1, 32]]),
                                in1=_ap(sw, 32, [[64, n], [1, 32]]), op=A.add)
                        nc.vector.tensor_tensor(
                            out=_ap(samp, t * 64, [[32, 2], [1, 32]]),
                            in0=_ap(sw, 0, [[64, 2], [1, 32]]),
                            in1=_ap(sw, 32, [[64, 2], [1, 32]]), op=A.add)

                    # output projection
                    sT = pB.tile([128, 2, 128], F32, tag="BsT")
                    for k in range(2):
                        tp = psB.tile([128, 128], F32, tag="Btp")
                        nc.tensor.transpose(tp[:], samp[:, k * 128:(k + 1) * 128], tid[:])
                        nc.scalar.copy(sT[:, k, :], tp[:])
                    o2p = psB.tile([128, D], F32, tag="Bmm")
                    nc.tensor.matmul(o2p[:], lhsT=sT[:, 0, :], rhs=tWout[:, 0:D], start=True, stop=False)
                    nc.tensor.matmul(o2p[:], lhsT=sT[:, 1, :], rhs=tWout[:, D:2 * D], start=False, stop=False)
                    nc.tensor.matmul(o2p[:], lhsT=tones[:], rhs=tbout[:], start=False, stop=True)

                    # residual + layernorm
                    def layer_norm(inp_sbuf, res_psum, gt, bt, tagp):
                        x1 = pB.tile([128, D], F32, tag=tagp + "x1")
                        sums = pB.tile([128, 1], F32, tag=tagp + "su")
                        nc.vector.scalar_tensor_tensor(
                            out=x1[:], in0=inp_sbuf[:], scalar=0.0, in1=res_psum[:],
                            op0=A.add, op1=A.add, accum_out=sums[:])
                        negm = pB.tile([128, 1], F32, tag=tagp + "nm")
                        nc.scalar.mul(negm[:], sums[:], -1.0 / D)
                        sq = pB.tile([128, D], F32, tag=tagp + "sq")
                        ssq = pB.tile([128, 1], F32, tag=tagp + "ss")
                        nc.scalar.activation(sq[:], x1[:], ACTF.Square,
                                             bias=negm[:, 0:1], accum_out=ssq[:])
                        sd = pB.tile([128, 1], F32, tag=tagp + "sd")
                        nc.scalar.activation(sd[:], ssq[:], ACTF.Sqrt,
                                             scale=1.0 / D, bias=teps[:, 0:1])
                        rstd = pB.tile([128, 1], F32, tag=tagp + "rs")
                        nc.vector.reciprocal(rstd[:], sd[:])
                        xh = pB.tile([128, D], F32, tag=tagp + "xh")
                        nc.vector.tensor_scalar(
                            out=xh[:], in0=x1[:], scalar1=negm[:, 0:1],
                            scalar2=rstd[:, 0:1], op0=A.add, op1=A.mult)
                        yv = pB.tile([128, D], F32, tag=tagp + "y")
                        nc.vector.tensor_tensor(out=yv[:], in0=xh[:], in1=gt[:], op=A.mult)
                        nc.vector.tensor_tensor(out=yv[:], in0=yv[:], in1=bt[:], op=A.add)
                        return yv

                    y1v = layer_norm(s, o2p, tg1, tbe1, "BL1")

                    # FFN
                    yT = pB.tile([128, 2, 128], F32, tag="ByT")
                    for k in range(2):
                        tp = psB.tile([128, 128], F32, tag="Btp")
                        nc.tensor.transpose(tp[:], y1v[:, k * 128:(k + 1) * 128], tid[:])
                        nc.scalar.copy(yT[:, k, :], tp[:])
                    h1 = pB1.tile([128, DFF], F32, tag="Bh1")
                    for j in range(8):
                        js = slice(j * 128, (j + 1) * 128)
                        hp = psB.tile([128, 128], F32, tag="Bhp")
                        nc.tensor.matmul(hp[:], lhsT=tW1[:, 0 * DFF + j * 128:0 * DFF + (j + 1) * 128],
                                         rhs=yT[:, 0, :], start=True, stop=False)
                        nc.tensor.matmul(hp[:], lhsT=tW1[:, 1 * DFF + j * 128:1 * DFF + (j + 1) * 128],
                                         rhs=yT[:, 1, :], start=False, stop=True)
                        nc.scalar.activation(h1[:, js], hp[:], ACTF.Relu,
                                             bias=tb1T[:, j:j + 1])
                    o3p = psB.tile([128, D], F32, tag="Bmm")
                    for j in range(8):
                        js = slice(j * 128, (j + 1) * 128)
                        nc.tensor.matmul(o3p[:], lhsT=h1[:, js], rhs=tW2[:, j * D:(j + 1) * D],
                                         start=(j == 0), stop=False)
                    nc.tensor.matmul(o3p[:], lhsT=tones[:], rhs=tb2[:], start=False, stop=True)

                    y2v = layer_norm(y1v, o3p, tg2, tbe2, "BL2")
                    nc.sync.dma_start(outq[rs, :], y2v[:])
                    if dbg:
                        nc.sync.dma_start(d_samp[rs, :], samp[:])

                # software pipeline: tile i+1's index/prep front-end is
                # emitted before tile i's consumers so DVE's gather waits
                # never starve GpSimd desc-gen of ready idx tables
                prev = None
                for i in range(0 if "nob" in ablate else N_Q_TILES):
                    cur = stage1(i)
                    if prev is not None:
                        stage2(prev)
                    prev = cur
                if prev is not None:
                    stage2(prev)

    nc.compile()
    return nc


# ----------------------------------------------------------------------
# host-side wrapper
# ----------------------------------------------------------------------
_NC_CACHE = None


def _get_nc():
    global _NC_CACHE
    if _NC_CACHE is None:
        _NC_CACHE = build()
    return _NC_CACHE


def make_consts():
    h_i, l_i, p_i = np.meshgrid(np.arange(NH), np.arange(NL), np.arange(NP), indexing="ij")
    Wl = np.array([w for (_, w) in SPATIAL], np.float32)
    Hl = np.array([h for (h, _) in SPATIAL], np.float32)
    lw = Wl[l_i].reshape(-1)
    lh = Hl[l_i].reshape(-1)
    base = ((h_i % 2) * STRIPE + FP
            + np.array(LEVEL_START, np.float32)[l_i] + 1).reshape(-1)
    rep = lambda v: np.tile(v[None, :].astype(np.float32), (128, 1))
    dims8 = np.zeros(NL * 2, np.float32)
    dims8[0::2] = Wl
    dims8[1::2] = Hl
    return {
        "cW": rep(lw), "cH": rep(lh),
        "cWm1": rep(lw - 1), "cHm1": rep(lh - 1),
        "cWm2": rep(lw - 2), "cHm2": rep(lh - 2),
        "cBASE": rep(base),
        "dims8": rep(dims8),
        "ident": np.eye(128, dtype=np.float32),
        "ones_row": np.ones((1, 128), np.float32),
    }


SHARD_STARTS = [0, 3324, 6648, 9972]
SHARD_SIZES = [3324, 3324, 3324, 3322]

_V2_ZEROS = None


def _v2_zeros():
    global _V2_ZEROS
    if _V2_ZEROS is None:
        _V2_ZEROS = np.zeros((NH * STRIPE, CHUNK), ml_dtypes.bfloat16)
    return _V2_ZEROS


def make_in_maps(inputs):
    consts = make_consts()
    in_maps = []
    for core in range(8):
        b, c = core // 4, core % 4
        st, sz = SHARD_STARTS[c], SHARD_SIZES[c]
        src_full = np.zeros((PAD_LEN, D), np.float32)
        src_full[:LEN] = inputs["src"][b]
        srcq = np.zeros((Q_SH, D), np.float32)
        srcq[:sz] = inputs["src"][b, st:st + sz]
        posq = np.zeros((Q_SH, D), np.float32)
        posq[:sz] = inputs["pos"][b, st:st + sz]
        refq = np.full((Q_SH, NL * 2), 0.5, np.float32)
        refq[:sz] = inputs["reference_points"][b, st:st + sz].reshape(sz, NL * 2)
        m = {
            "src_full": src_full, "srcq": srcq, "posq": posq, "refq": refq,
            "Woa": np.concatenate([inputs["W_off"], inputs["W_attn"]], axis=1),
            "Wout": inputs["W_out"],
            "W1": inputs["W1"], "W2": inputs["W2"],
            "boa": np.concatenate([inputs["b_off"], inputs["b_attn"]])[None, :],
            "bout": inputs["b_out"][None, :],
            "b1T": np.asarray(inputs["b1"]).reshape(DFF // 128, 128).T,
            "b2": inputs["b2"][None, :],
            "g1r": np.tile(inputs["g1"][None, :], (128, 1)),
            "be1r": np.tile(inputs["be1"][None, :], (128, 1)),
            "g2r": np.tile(inputs["g2"][None, :], (128, 1)),
            "be2r": np.tile(inputs["be2"][None, :], (128, 1)),
        }
        for k in ("cW", "cH", "cWm1", "cHm1", "cWm2", "cHm2", "cBASE", "dims8", "ident", "ones_row"):
            m[k] = consts[k]
        m = {k: np.ascontiguousarray(v, np.float32) for k, v in m.items()}
        m["Wv"] = np.ascontiguousarray(inputs["W_value"], ml_dtypes.bfloat16)
        m["bv"] = np.ascontiguousarray(
            np.asarray(inputs["b_value"])[None, :], ml_dtypes.bfloat16)
        m["value2"] = _v2_zeros()
        in_maps.append(m)
    return in_maps


def assemble_out(results):
    out = np.empty((2, LEN, D), np.float32)
    for core in range(8):
        b, c = core // 4, core % 4
        st, sz = SHARD_STARTS[c], SHARD_SIZES[c]
        out[b, st:st + sz] = results[core]["outq"][:sz]
    return out


def run(inputs, trace=False, **kw):
    nc = _get_nc()
    in_maps = make_in_maps(inputs)
    res = run_bass_kernel_spmd(nc, in_maps, core_ids=list(range(8)), trace=trace, **kw)
    return assemble_out(res.results), res


def kernel(**inputs):
    out, _ = run(inputs)
    return out

